# revision 2
# baseline (speedup 1.0000x reference)
"""GCN (gnn_message_passing) Trainium2 kernel, 8 NeuronCores.

Math: with IN_F=1, h = x @ W1 is rank-1, so the whole GCNConv collapses to
per-node scalars:
    deg[n]  = 1 + indegree(n)
    dinv[n] = 1/sqrt(deg[n])
    p[n]    = x[n] * dinv[n]
    q[n]    = sum_{e: dst=n} p[src_e]
    s[n]    = dinv[n] * (p[n] + q[n])          # includes self loop
    t_g     = mean_{n in g} s[n]
    logits  = t_g * (W1@W2) + (b1@W2 + b2); log_softmax rows.

Device plan (per core):
  - edges range-sharded by dst into 16 shards (2 per core).  Host orders each
    shard's edges by (src-subtable, subrank-within-(dst,subtable)) so that
    every dma_scatter_add call has unique destination indices (the HW
    scatter-add loses updates / crashes on duplicate indices in one call).
  - deg histogram + q accumulation: SBUF-destination dma_scatter_add calls,
    one per (shard, subtable, subrank-class).
  - p[src] gather: AllGather p, build a stride-64 "fat" table T[n][0]=p[n]
    in DRAM, then MoE dma_gather (256B rows, int16 row idx per 30720-row
    subtable), extract column 0.
  - pooling: one-hot matmuls (PSUM accumulate) over the dense s tile.
  - AllReduce tiny per-graph (sum, count); dense tail -> [G, 10] logits.
"""

import sys
for _p in ("/opt/trn_rl_repo", "/root/.axon_site/_ro/trn_rl_repo"):
    if _p not in sys.path:
        sys.path.insert(0, _p)

from dataclasses import dataclass, field

import numpy as np

import concourse.bacc as bacc
import concourse.bass as bass
import concourse.mybir as mybir
import concourse.tile as tile
from concourse import bass_utils

P = 128


@dataclass(frozen=True)
class Cfg:
    N: int = 307200          # nodes
    E: int = 5734400         # edges
    G: int = 4096            # graphs
    CLS: int = 10
    HID: int = 64
    NC: int = 8              # cores
    NSUB: int = 10           # gather subtables (rows must fit int16)
    GCH: int = 8192          # gather call chunk (slots)
    # static per-(shard,subtable) scatter-call schedule: slots per subrank
    # class j (must cover the actual data; padded slots go to a trash slot)
    SCHED: tuple = (16640, 11008, 5888, 2560, 1024, 512,
                    128, 128, 128, 128, 128, 128, 128)

    @property
    def RC(self):            # nodes per core
        return self.N // self.NC

    @property
    def RS(self):            # nodes per shard (2 shards per core)
        return self.RC // 2

    @property
    def COLS(self):          # free columns of per-core dense node tiles
        return self.RC // P

    @property
    def SCOLS(self):
        return self.RS // P

    @property
    def PGRP(self):          # parity-table groups per shard
        return self.RS // 256

    @property
    def GCOL(self):
        return self.G // P

    @property
    def SUBROWS(self):
        return self.N // self.NSUB

    @property
    def SBT(self):           # slots per (shard, subtable)
        return sum(self.SCHED)

    @property
    def TOT(self):           # slots per core
        return 2 * self.NSUB * self.SBT


def _scatter_calls(cfg):
    """Per-(shard,subtable) scatter call sizes: SCHED classes split to
    <=4096 indices per call (larger calls overflow the SWDGE ring)."""
    calls = []
    for n in cfg.SCHED:
        left = n
        while left > 0:
            c = min(4096, left)
            calls.append(c)
            left -= c
    return calls


def _gather_chunks(cfg):
    """Chunk sizes for one (shard, subtable) region's gather calls."""
    out, left = [], cfg.SBT
    while left > 0:
        c = min(cfg.GCH, left)
        out.append(c)
        left -= c
    return out


def build_nc(cfg: Cfg):
    f32 = mybir.dt.float32
    i16 = mybir.dt.int16
    nc = bacc.Bacc("TRN2", target_bir_lowering=False, debug=False)

    x_loc = nc.dram_tensor("x_loc", [P, cfg.COLS], f32, kind="ExternalInput")
    dstw = nc.dram_tensor("dstw", [P, cfg.TOT // 16], i16, kind="ExternalInput")
    srcw = nc.dram_tensor("srcw", [P, cfg.TOT // 16], i16, kind="ExternalInput")
    plo = nc.dram_tensor("plo", [P, cfg.COLS], f32, kind="ExternalInput")
    phi = nc.dram_tensor("phi", [P, cfg.COLS], f32, kind="ExternalInput")
    iota128 = nc.dram_tensor("iota128", [P, P], f32, kind="ExternalInput")
    iotagc = nc.dram_tensor("iotagc", [P, cfg.GCOL], f32, kind="ExternalInput")
    W1d = nc.dram_tensor("W1", [1, cfg.HID], f32, kind="ExternalInput")
    b1d = nc.dram_tensor("b1", [cfg.HID], f32, kind="ExternalInput")
    W2d = nc.dram_tensor("W2", [cfg.HID, cfg.CLS], f32, kind="ExternalInput")
    b2d = nc.dram_tensor("b2", [cfg.CLS], f32, kind="ExternalInput")
    out_d = nc.dram_tensor("out", [cfg.G, cfg.CLS], f32, kind="ExternalOutput")

    Tfat = nc.dram_tensor("Tfat", [cfg.N, 64], f32, kind="Internal")

    chunks = _gather_chunks(cfg)

    with tile.TileContext(nc) as tc:
        with (
            tc.tile_pool(name="state", bufs=1) as st,
            tc.tile_pool(name="sbuf", bufs=3) as sb,
            tc.tile_pool(name="psum", bufs=1, space="PSUM") as ps,
            tc.tile_pool(name="dram", bufs=1, space="DRAM") as dr,
        ):
            TG = cfg.PGRP + 1  # +1 trash group for padding slots
            deg_t = [st.tile([P, TG], f32, tag=f"deg{i}", name=f"deg{i}")
                     for i in range(4)]
            qg_t = [st.tile([P, TG], f32, tag=f"qg{i}", name=f"qg{i}")
                    for i in range(4)]
            for t in deg_t + qg_t:
                nc.vector.memzero(t[:])

            maxsch = max(cfg.SCHED)
            ones_val = st.tile([P, maxsch // P, 1], f32)
            nc.vector.memset(ones_val[:], 1.0)

            # ---- phase 1: degree histogram (scatter ones per call) ----
            scalls = _scatter_calls(cfg)
            off = 0
            for s in (0, 1):
                for b in range(cfg.NSUB):
                    for n in scalls:
                        it = sb.tile([P, n // 16], i16, tag="idx")
                        nc.sync.dma_start(
                            out=it[:], in_=dstw[:, off // 16:(off + n) // 16])
                        nc.gpsimd.dma_scatter_add(
                            deg_t[2 * s][:], ones_val[:, :n // P, :], it[:],
                            n, n, 1,
                            sbuf_tokens_per_rank=P, parity_reg=0,
                            out_ap_other=deg_t[2 * s + 1][:])
                        off += n

            # ---- phase 2: dinv, p ----
            deg_d = st.tile([P, cfg.COLS], f32)
            for s in (0, 1):
                for par in (0, 1):
                    nc.vector.tensor_copy(
                        out=deg_d[:, s * cfg.SCOLS + par:
                                  s * cfg.SCOLS + 2 * cfg.PGRP:2],
                        in_=deg_t[2 * s + par][:, :cfg.PGRP])
            sqd = sb.tile([P, cfg.COLS], f32)
            nc.scalar.activation(sqd[:], deg_d[:],
                                 mybir.ActivationFunctionType.Sqrt, bias=1.0)
            dinv = st.tile([P, cfg.COLS], f32)
            nc.vector.reciprocal(dinv[:], sqd[:])
            xt = sb.tile([P, cfg.COLS], f32)
            nc.sync.dma_start(out=xt[:], in_=x_loc[:])
            p_t = st.tile([P, cfg.COLS], f32)
            nc.vector.tensor_tensor(out=p_t[:], in0=xt[:], in1=dinv[:],
                                    op=mybir.AluOpType.mult)

            # ---- phase 3: AllGather p; build fat table ----
            p_in = dr.tile([P, cfg.COLS], f32)
            p_full = dr.tile([cfg.NC * P, cfg.COLS], f32)
            nc.gpsimd.dma_start(out=p_in[:], in_=p_t[:])
            nc.gpsimd.collective_compute(
                "AllGather", mybir.AluOpType.bypass,
                replica_groups=[list(range(cfg.NC))],
                ins=[p_in[:].opt()], outs=[p_full[:].opt()])
            for i in range(cfg.NC):
                nc.sync.dma_start(
                    out=Tfat[i * cfg.RC:(i + 1) * cfg.RC, 0:1],
                    in_=p_full[i * P:(i + 1) * P, :].rearrange(
                        "c (b o) -> (c b) o", o=1))

            # ---- phase 4: gather p[src] into W (per shard, subtable) ----
            W = st.tile([P, cfg.TOT // P, 1], f32)
            goff = 0
            for s in (0, 1):
                for b in range(cfg.NSUB):
                    for cn in chunks:
                        gi = sb.tile([P, cfg.GCH // 16], i16, tag="gidx")
                        nc.sync.dma_start(
                            out=gi[:, :cn // 16],
                            in_=srcw[:, goff // 16:(goff + cn) // 16])
                        go = sb.tile([P, cfg.GCH // P, 64], f32, tag="gout")
                        nc.gpsimd.dma_gather(
                            go[:, :cn // P, :],
                            Tfat[b * cfg.SUBROWS:(b + 1) * cfg.SUBROWS, :],
                            gi[:, :cn // 16],
                            cn, cn, 64, single_packet=False)
                        nc.vector.tensor_copy(
                            out=W[:, goff // P:(goff + cn) // P, 0],
                            in_=go[:, :cn // P, 0])
                        goff += cn

            # ---- phase 5: scatter-add W into q ----
            off = 0
            for s in (0, 1):
                for b in range(cfg.NSUB):
                    for n in scalls:
                        it = sb.tile([P, n // 16], i16, tag="idx")
                        nc.sync.dma_start(
                            out=it[:], in_=dstw[:, off // 16:(off + n) // 16])
                        nc.gpsimd.dma_scatter_add(
                            qg_t[2 * s][:], W[:, off // P:(off + n) // P, :],
                            it[:], n, n, 1,
                            sbuf_tokens_per_rank=P, parity_reg=0,
                            out_ap_other=qg_t[2 * s + 1][:])
                        off += n

            # ---- phase 6: s = dinv * (p + q); pool via one-hot matmuls ----
            qg_d = sb.tile([P, cfg.COLS], f32)
            for s in (0, 1):
                for par in (0, 1):
                    nc.vector.tensor_copy(
                        out=qg_d[:, s * cfg.SCOLS + par:
                                 s * cfg.SCOLS + 2 * cfg.PGRP:2],
                        in_=qg_t[2 * s + par][:, :cfg.PGRP])
            s_d = sb.tile([P, cfg.COLS], f32)
            nc.vector.tensor_tensor(out=s_d[:], in0=p_t[:], in1=qg_d[:],
                                    op=mybir.AluOpType.add)
            nc.vector.tensor_tensor(out=s_d[:], in0=s_d[:], in1=dinv[:],
                                    op=mybir.AluOpType.mult)

            io128 = st.tile([P, P], f32)
            nc.sync.dma_start(out=io128[:], in_=iota128[:])
            iogc = st.tile([P, cfg.GCOL], f32)
            nc.sync.dma_start(out=iogc[:], in_=iotagc[:])
            plo_t = st.tile([P, cfg.COLS], f32)
            nc.sync.dma_start(out=plo_t[:], in_=plo[:])
            phi_t = st.tile([P, cfg.COLS], f32)
            nc.sync.dma_start(out=phi_t[:], in_=phi[:])
            ps_cnt = ps.tile([P, cfg.GCOL], f32, tag="pscnt")
            ps_sum = ps.tile([P, cfg.GCOL], f32, tag="pssum")
            for t in range(cfg.COLS):
                oh_lo = sb.tile([P, P], f32, tag="ohlo")
                nc.vector.tensor_scalar(
                    out=oh_lo[:], in0=io128[:], scalar1=plo_t[:, t:t + 1],
                    scalar2=None, op0=mybir.AluOpType.is_equal)
                oh_s = sb.tile([P, P], f32, tag="ohs")
                nc.vector.tensor_scalar(
                    out=oh_s[:], in0=oh_lo[:], scalar1=s_d[:, t:t + 1],
                    scalar2=None, op0=mybir.AluOpType.mult)
                oh_hi = sb.tile([P, cfg.GCOL], f32, tag="ohhi")
                nc.vector.tensor_scalar(
                    out=oh_hi[:], in0=iogc[:], scalar1=phi_t[:, t:t + 1],
                    scalar2=None, op0=mybir.AluOpType.is_equal)
                nc.tensor.matmul(ps_cnt[:], lhsT=oh_lo[:], rhs=oh_hi[:],
                                 start=(t == 0), stop=(t == cfg.COLS - 1))
                nc.tensor.matmul(ps_sum[:], lhsT=oh_s[:], rhs=oh_hi[:],
                                 start=(t == 0), stop=(t == cfg.COLS - 1))

            # ---- phase 7: AllReduce (sum, count) ----
            g_d = sb.tile([P, cfg.GCOL, 2], f32)
            nc.vector.tensor_copy(out=g_d[:, :, 0:1],
                                  in_=ps_sum[:].rearrange("p (a o) -> p a o", o=1))
            nc.vector.tensor_copy(out=g_d[:, :, 1:2],
                                  in_=ps_cnt[:].rearrange("p (a o) -> p a o", o=1))
            r_in = dr.tile([P, cfg.GCOL * 2], f32)
            r_out = dr.tile([P, cfg.GCOL * 2], f32)
            nc.gpsimd.dma_start(out=r_in[:],
                                in_=g_d[:].rearrange("p a b -> p (a b)"))
            nc.gpsimd.collective_compute(
                "AllReduce", mybir.AluOpType.add,
                replica_groups=[list(range(cfg.NC))],
                ins=[r_in[:].opt()], outs=[r_out[:].opt()])
            sg = sb.tile([P, cfg.GCOL, 2], f32)
            nc.sync.dma_start(out=sg[:].rearrange("p a b -> p (a b)"),
                              in_=r_out[:])

            # ---- phase 8: v = W1@W2, u = b1@W2 + b2 (broadcast to 128) ----
            w1t = sb.tile([cfg.HID, 1], f32)
            nc.sync.dma_start(out=w1t[:], in_=W1d[:].rearrange("o k -> k o"))
            b1t = sb.tile([cfg.HID, 1], f32)
            nc.sync.dma_start(out=b1t[:],
                              in_=b1d[:].rearrange("(k o) -> k o", o=1))
            w2t = sb.tile([cfg.HID, cfg.CLS], f32)
            nc.sync.dma_start(out=w2t[:], in_=W2d[:])
            b2t = sb.tile([1, cfg.CLS], f32)
            nc.sync.dma_start(out=b2t[:],
                              in_=b2d[:].rearrange("(o k) -> o k", o=1))
            pv1 = ps.tile([1, cfg.CLS], f32, tag="pv1")
            nc.tensor.matmul(pv1[:], lhsT=w1t[:], rhs=w2t[:],
                             start=True, stop=True)
            pu1 = ps.tile([1, cfg.CLS], f32, tag="pu1")
            nc.tensor.matmul(pu1[:], lhsT=b1t[:], rhs=w2t[:],
                             start=True, stop=True)
            v1 = sb.tile([1, cfg.CLS], f32)
            nc.vector.tensor_copy(out=v1[:], in_=pv1[:])
            u1 = sb.tile([1, cfg.CLS], f32)
            nc.vector.tensor_tensor(out=u1[:], in0=pu1[:], in1=b2t[:],
                                    op=mybir.AluOpType.add)
            ones_row = sb.tile([1, P], f32)
            nc.vector.memset(ones_row[:], 1.0)
            pvb = ps.tile([P, cfg.CLS], f32, tag="pvb")
            nc.tensor.matmul(pvb[:], lhsT=ones_row[:], rhs=v1[:],
                             start=True, stop=True)
            pub = ps.tile([P, cfg.CLS], f32, tag="pub")
            nc.tensor.matmul(pub[:], lhsT=ones_row[:], rhs=u1[:],
                             start=True, stop=True)
            vb = sb.tile([P, cfg.CLS], f32)
            nc.vector.tensor_copy(out=vb[:], in_=pvb[:])
            ub = sb.tile([P, cfg.CLS], f32)
            nc.vector.tensor_copy(out=ub[:], in_=pub[:])

            # ---- phase 9: t = S / max(cnt,1); logits; log_softmax ----
            cntc = sb.tile([P, cfg.GCOL], f32)
            nc.vector.tensor_scalar(out=cntc[:], in0=sg[:, :, 1],
                                    scalar1=1.0, scalar2=None,
                                    op0=mybir.AluOpType.max)
            rcp = sb.tile([P, cfg.GCOL], f32)
            nc.vector.reciprocal(rcp[:], cntc[:])
            tg = sb.tile([P, cfg.GCOL], f32)
            nc.vector.tensor_tensor(out=tg[:], in0=sg[:, :, 0], in1=rcp[:],
                                    op=mybir.AluOpType.mult)
            L = sb.tile([P, cfg.GCOL, cfg.CLS], f32)
            for c in range(cfg.CLS):
                nc.vector.tensor_scalar(
                    out=L[:, :, c], in0=tg[:],
                    scalar1=vb[:, c:c + 1], scalar2=ub[:, c:c + 1],
                    op0=mybir.AluOpType.mult, op1=mybir.AluOpType.add)
            m = sb.tile([P, cfg.GCOL], f32)
            nc.vector.tensor_reduce(out=m[:], in_=L[:],
                                    axis=mybir.AxisListType.X,
                                    op=mybir.AluOpType.max)
            Lm = sb.tile([P, cfg.GCOL, cfg.CLS], f32)
            nc.vector.tensor_tensor(
                out=Lm[:], in0=L[:],
                in1=m[:].to_broadcast([P, cfg.GCOL, cfg.CLS]),
                op=mybir.AluOpType.subtract)
            ex = sb.tile([P, cfg.GCOL, cfg.CLS], f32)
            nc.scalar.activation(ex[:], Lm[:],
                                 mybir.ActivationFunctionType.Exp)
            se = sb.tile([P, cfg.GCOL], f32)
            nc.vector.tensor_reduce(out=se[:], in_=ex[:],
                                    axis=mybir.AxisListType.X,
                                    op=mybir.AluOpType.add)
            ls = sb.tile([P, cfg.GCOL], f32)
            nc.scalar.activation(ls[:], se[:],
                                 mybir.ActivationFunctionType.Ln)
            outt = sb.tile([P, cfg.GCOL, cfg.CLS], f32)
            nc.vector.tensor_tensor(
                out=outt[:], in0=Lm[:],
                in1=ls[:].to_broadcast([P, cfg.GCOL, cfg.CLS]),
                op=mybir.AluOpType.subtract)
            nc.sync.dma_start(
                out=out_d[:].rearrange("(c p) k -> p c k", p=P),
                in_=outt[:])

    nc.compile()
    return nc


def _wrap16(a):
    """[n] -> [128, n//16] wrapped-by-16 layout replicated across the 8
    GPSIMD cores' partition groups."""
    return np.ascontiguousarray(np.tile(a.reshape(-1, 16).T, (8, 1)))


def prep_inputs(cfg: Cfg, x, edge_index, batch, W1, b1, W2, b2):
    x = np.asarray(x, np.float32).reshape(-1)
    ei = np.asarray(edge_index)
    batch = np.asarray(batch).astype(np.int64)
    src = ei[0].astype(np.int64)
    dst = ei[1].astype(np.int64)

    # transformed gather table index: fat table rows follow the AllGather'ed
    # p layout: row = c*RC + (r%128)*COLS + r//128 for global node c*RC + r
    c_of = src // cfg.RC
    r_of = src % cfg.RC
    srcT = c_of * cfg.RC + (r_of % P) * cfg.COLS + r_of // P

    shard = dst // cfg.RS                    # 0..15
    sub = srcT // cfg.SUBROWS                # 0..NSUB-1
    # subrank: rank of the edge within its (dst, subtable) group
    key = dst * cfg.NSUB + sub
    o1 = np.argsort(key, kind="stable")
    k_sorted = key[o1]
    grp_start = np.zeros(cfg.N * cfg.NSUB + 1, np.int64)
    cnts = np.bincount(k_sorted, minlength=cfg.N * cfg.NSUB)
    np.cumsum(cnts, out=grp_start[1:])
    subrank = np.empty(cfg.E, np.int64)
    subrank[o1] = np.arange(cfg.E) - grp_start[k_sorted]
    J = len(cfg.SCHED)
    assert subrank.max() < J, f"subrank {subrank.max()} >= {J}"

    # final order: (shard, subtable, subrank); count per call
    callkey = (shard * cfg.NSUB + sub) * J + subrank
    o2 = np.argsort(callkey, kind="stable")
    ck_sorted = callkey[o2]
    ncalls = 16 * cfg.NSUB * J
    ccnt = np.bincount(ck_sorted, minlength=ncalls).reshape(16, cfg.NSUB, J)
    sched = np.asarray(cfg.SCHED)
    assert (ccnt <= sched[None, None, :]).all(), (
        ccnt.max(axis=(0, 1)), cfg.SCHED)

    cstart = np.zeros(ncalls + 1, np.int64)
    np.cumsum(ccnt.reshape(-1), out=cstart[1:])

    dl_all = (dst - shard * cfg.RS)[o2].astype(np.int16)
    sr_all = (srcT - sub * cfg.SUBROWS)[o2].astype(np.int16)

    iota128 = np.tile(np.arange(P, dtype=np.float32), (P, 1))
    iotagc = np.tile(np.arange(cfg.GCOL, dtype=np.float32), (P, 1))

    in_maps = []
    for c in range(cfg.NC):
        dslot = (cfg.RS + np.arange(cfg.TOT) % 256).astype(np.int16)
        sslot = np.zeros(cfg.TOT, np.int16)          # pad gathers row 0
        off = 0
        for s2 in (0, 1):
            sh = 2 * c + s2
            for b in range(cfg.NSUB):
                for j in range(J):
                    ci = (sh * cfg.NSUB + b) * J + j
                    n = ccnt[sh, b, j]
                    dslot[off:off + n] = dl_all[cstart[ci]:cstart[ci] + n]
                    sslot[off:off + n] = sr_all[cstart[ci]:cstart[ci] + n]
                    off += cfg.SCHED[j]
        nl = np.arange(cfg.RC)
        gn = c * cfg.RC + nl                          # dense tile n = col*128+p
        xl = x[gn].reshape(cfg.COLS, P).T
        bat = batch[gn].reshape(cfg.COLS, P).T
        in_maps.append({
            "x_loc": np.ascontiguousarray(xl),
            "dstw": _wrap16(dslot),
            "srcw": _wrap16(sslot),
            "plo": np.ascontiguousarray((bat % P).astype(np.float32)),
            "phi": np.ascontiguousarray((bat // P).astype(np.float32)),
            "iota128": iota128,
            "iotagc": iotagc,
            "W1": np.asarray(W1, np.float32).reshape(1, cfg.HID),
            "b1": np.asarray(b1, np.float32).reshape(cfg.HID),
            "W2": np.asarray(W2, np.float32).reshape(cfg.HID, cfg.CLS),
            "b2": np.asarray(b2, np.float32).reshape(cfg.CLS),
        })
    return in_maps


def build_noop(cfg: Cfg):
    """Same I/O signature, trivial device work — isolates host overhead."""
    f32 = mybir.dt.float32
    i16 = mybir.dt.int16
    nc = bacc.Bacc("TRN2", target_bir_lowering=False, debug=False)
    nc.dram_tensor("x_loc", [P, cfg.COLS], f32, kind="ExternalInput")
    nc.dram_tensor("dstw", [P, cfg.TOT // 16], i16, kind="ExternalInput")
    nc.dram_tensor("srcw", [P, cfg.TOT // 16], i16, kind="ExternalInput")
    nc.dram_tensor("plo", [P, cfg.COLS], f32, kind="ExternalInput")
    nc.dram_tensor("phi", [P, cfg.COLS], f32, kind="ExternalInput")
    nc.dram_tensor("iota128", [P, P], f32, kind="ExternalInput")
    nc.dram_tensor("iotagc", [P, cfg.GCOL], f32, kind="ExternalInput")
    nc.dram_tensor("W1", [1, cfg.HID], f32, kind="ExternalInput")
    nc.dram_tensor("b1", [cfg.HID], f32, kind="ExternalInput")
    nc.dram_tensor("W2", [cfg.HID, cfg.CLS], f32, kind="ExternalInput")
    nc.dram_tensor("b2", [cfg.CLS], f32, kind="ExternalInput")
    out_d = nc.dram_tensor("out", [cfg.G, cfg.CLS], f32,
                           kind="ExternalOutput")
    with tile.TileContext(nc) as tc:
        with tc.tile_pool(name="sbuf", bufs=1) as sb:
            z = sb.tile([P, cfg.GCOL, cfg.CLS], f32)
            nc.vector.memzero(z[:])
            nc.sync.dma_start(
                out=out_d[:].rearrange("(c p) k -> p c k", p=P), in_=z[:])
    nc.compile()
    return nc


_NC_CACHE = {}


def _get_nc(cfg: Cfg):
    if cfg not in _NC_CACHE:
        _NC_CACHE[cfg] = build_nc(cfg)
    return _NC_CACHE[cfg]


def run(cfg: Cfg, inputs, **run_kwargs):
    nc = _get_nc(cfg)
    in_maps = prep_inputs(cfg, **inputs)
    res = bass_utils.run_bass_kernel_spmd(
        nc, in_maps, core_ids=list(range(cfg.NC)), **run_kwargs)
    return res


def kernel(x, edge_index, batch, W1, b1, W2, b2):
    cfg = Cfg()
    res = run(cfg, dict(x=x, edge_index=edge_index, batch=batch,
                        W1=W1, b1=b1, W2=W2, b2=b2))
    return res.results[0]["out"]



# revision 5
# speedup vs baseline: 73.7736x; 73.7736x over previous
"""GCN (gnn_message_passing) Trainium2 kernel v2, 8 NeuronCores.

Math (IN_F=1 makes GCNConv rank-1; per-node scalars):
    deg[n]  = 1 + global indegree(n)        (host, structure-only)
    dinv[n] = 1/sqrt(deg[n])                (host, structure-only)
    p[n]    = x[n] * dinv[n]                (device)
    q[n]    = sum_{e: dst=n} p[src_e]       (device scatter-add stream)
    s[n]    = dinv[n] * (p[n] + q[n])       (device; self loop included)
    t_g     = mean_{n in g} s[n]            (device one-hot matmul pooling)
    logits  = t_g*(W1@W2) + (b1@W2+b2); log_softmax.   (device)

Scatter-only message passing (no per-edge gather at all):
  - Nodes are dst-range sharded: core c owns [c*38400, (c+1)*38400), split
    into 2 shards of 19200 (dst slot ids must fit int16).
  - For each (core, shard), the host groups sources by out-degree k into
    that shard (k>=4 split into parts of <=3; multi-edges and k>=7 tails go
    to an overflow pool) and packs them into "windows" of W sources such
    that ALL dsts in a window are distinct (randomized greedy).
  - x and dinv are uploaded in this per-shard source order sigma (each
    source at most once per (core,shard,group) + tiny overflow dup).
    Device computes p_sigma = x_sigma*dinv_sigma, then a free DVE broadcast
    "expand" replicates each window's p column block K times (K = group k);
    plane r of the window holds each source's r-th edge.
  - One dma_scatter_add per window (num_idxs = K*W, all dsts distinct)
    accumulates into per-shard parity-split SBUF q tiles, round-robin over
    2 accumulator pairs and 4 SWDGE queues for pipelining.
  - Overflow pool is scheduled as levels of (unique-src, unique-dst) edges,
    k=1 style.
  - Pooling/AllReduce/tail as in v1 (one-hot matmuls, [G,10] logits).
"""

import sys
for _p in ("/opt/trn_rl_repo", "/root/.axon_site/_ro/trn_rl_repo"):
    if _p not in sys.path:
        sys.path.insert(0, _p)

from dataclasses import dataclass

import numpy as np

import concourse.bacc as bacc
import concourse.mybir as mybir
import concourse.tile as tile
from concourse import bass_utils

P = 128


@dataclass(frozen=True)
class Cfg:
    N: int = 307200          # nodes
    E: int = 5734400         # edges
    G: int = 4096            # graphs
    CLS: int = 10
    HID: int = 64
    NC: int = 8              # cores
    RS: int = 19200          # nodes per shard (2 shards per core)
    TRASH: int = 19456       # trash slot base (distinct per call position)
    QCOLS: int = 112         # q tile cols: covers idx<=27647 (g<=107)
    R_ACC: int = 4           # accumulator pairs per shard
    NQ: int = 4              # SWDGE queues
    # static per-shard window schedule: (W sources, K planes) per window
    K1W: int = 8064
    K1N: int = 15
    K2W: int = 3968
    K2N: int = 22
    K3W: int = 2688
    K3N: int = 11
    OVF: tuple = (2560, 1536, 1024, 512, 256, 256, 128, 128)

    @property
    def RC(self):
        return 2 * self.RS

    @property
    def COLS(self):          # free cols of dense per-core node tiles
        return self.RC // P

    @property
    def GCOL(self):
        return self.G // P

    @property
    def SCHED(self):
        s = [(self.K1W, 1)] * self.K1N + [(self.K2W, 2)] * self.K2N \
            + [(self.K3W, 3)] * self.K3N + [(w, 1) for w in self.OVF]
        return tuple(s)

    @property
    def SIGCOLS(self):       # sigma cols per shard
        return sum(w // P for w, _ in self.SCHED)

    @property
    def SLOTS(self):         # idx slots per shard
        return sum(w * k for w, k in self.SCHED)


# ---------------------------------------------------------------- planner

def _assign_windows(D, nW, cap, rng, rounds=24):
    """D [n, k] dst-local ids. Place each row into one of nW windows of
    capacity cap with all dsts distinct within a window (across rows and
    planes). Returns win [n] (-1 = failed)."""
    n, k = D.shape
    RS = 19200
    win = np.full(n, -1, np.int64)
    fill = np.zeros(nW, np.int64)
    used = np.zeros(nW * RS, bool)
    pending = rng.permutation(n)
    for _ in range(rounds):
        if len(pending) == 0:
            break
        cand = rng.integers(0, nW, len(pending))
        notfull = fill[cand] < cap
        cand2 = rng.integers(0, nW, len(pending))
        cand = np.where(notfull, cand, cand2)
        keys = (cand[:, None] * RS + D[pending]).ravel()
        uniq, first = np.unique(keys, return_index=True)
        pos = np.searchsorted(uniq, keys)
        is_first = first[pos] == np.arange(len(keys))
        ok = (is_first & ~used[keys]).reshape(len(pending), k).all(1)
        # capacity: rank within window among batch + fill < cap
        sel_rows = np.flatnonzero(ok)
        if len(sel_rows):
            sw = cand[sel_rows]
            o = np.argsort(sw, kind="stable")
            ss = sw[o]
            gf = np.ones(len(ss), bool)
            gf[1:] = ss[1:] != ss[:-1]
            firsts = np.flatnonzero(gf)
            gidx = np.cumsum(gf) - 1
            rank = np.arange(len(ss)) - firsts[gidx]
            keep = np.empty(len(sw), bool)
            keep[o] = rank + fill[ss] < cap
            ok[sel_rows[~keep]] = False
        sel = pending[ok]
        wsel = cand[ok]
        win[sel] = wsel
        used[(wsel[:, None] * RS + D[sel]).ravel()] = True
        np.add.at(fill, wsel, 1)
        pending = pending[~ok]
    return win


def plan_shard(src_sh, dl_sh, rng, cfg: Cfg):
    """Schedule one (core, shard)'s edges.

    Returns (sigma [SIGCOLS*128] int64 source ids (-1 pad),
             idx   [SLOTS] int16 dst slots)."""
    RS = cfg.RS
    # ---- dedup multi-edges: keep first (s,d), rest -> overflow
    key = src_sh * RS + dl_sh
    order = np.argsort(key, kind="stable")
    ks = key[order]
    fs = np.ones(len(ks), bool)
    fs[1:] = ks[1:] != ks[:-1]
    main_s, main_d = src_sh[order][fs], dl_sh[order][fs]
    ovf_s = [src_sh[order][~fs]]
    ovf_d = [dl_sh[order][~fs]]

    k_of = np.bincount(main_s, minlength=cfg.N)
    o2 = np.argsort(main_s, kind="stable")
    dsort = main_d[o2]
    off = np.zeros(cfg.N + 1, np.int64)
    np.cumsum(k_of, out=off[1:])

    # ---- build part lists per class g: S_g sources, D_g [m, g] dsts
    parts_S = {1: [], 2: [], 3: []}
    parts_D = {1: [], 2: [], 3: []}
    SPLIT = {1: (1,), 2: (2,), 3: (3,), 4: (2, 2), 5: (3, 2), 6: (3, 3)}
    for k in range(1, 7):
        S = np.flatnonzero(k_of == k)
        if not len(S):
            continue
        D = dsort[off[S][:, None] + np.arange(k)[None, :]]
        c0 = 0
        for g in SPLIT[k]:
            parts_S[g].append(S)
            parts_D[g].append(D[:, c0:c0 + g])
            c0 += g
    S = np.flatnonzero(k_of > 6)
    for s in S:
        ovf_s.append(np.full(k_of[s], s))
        ovf_d.append(dsort[off[s]:off[s + 1]])

    # ---- greedy window assignment per class
    sigma = np.full(cfg.SIGCOLS * P, -1, np.int64)
    idx = np.zeros(cfg.SLOTS, np.int16)
    sched = cfg.SCHED
    # window slot ranges
    sig_base = np.zeros(len(sched), np.int64)
    idx_base = np.zeros(len(sched), np.int64)
    sb = ib = 0
    for i, (w, k) in enumerate(sched):
        sig_base[i], idx_base[i] = sb, ib
        sb += w
        ib += w * k
    win_of_class = {
        1: list(range(0, cfg.K1N)),
        2: list(range(cfg.K1N, cfg.K1N + cfg.K2N)),
        3: list(range(cfg.K1N + cfg.K2N, cfg.K1N + cfg.K2N + cfg.K3N)),
    }
    ovf_wins = list(range(cfg.K1N + cfg.K2N + cfg.K3N, len(sched)))

    def fill_window(wi, S_sel, D_sel):
        """Place rows (sources S_sel with dsts D_sel [m,g]) into window wi."""
        w, k = sched[wi]
        m = len(S_sel)
        assert m <= w and D_sel.shape[1] == k
        sigma[sig_base[wi]:sig_base[wi] + m] = S_sel
        blk = np.full((k, w), -1, np.int64)
        blk[:, :m] = D_sel.T
        pad = blk < 0
        blk[pad] = cfg.TRASH + (np.arange(k * w).reshape(k, w))[pad] % 13312
        idx[idx_base[wi]:idx_base[wi] + w * k] = blk.reshape(-1)

    for g in (1, 2, 3):
        if not parts_S[g]:
            for wi in win_of_class[g]:
                fill_window(wi, np.zeros(0, np.int64),
                            np.zeros((0, g), np.int64))
            continue
        S_g = np.concatenate(parts_S[g])
        D_g = np.concatenate(parts_D[g])
        wins = win_of_class[g]
        nW, cap = len(wins), sched[wins[0]][0]
        if len(S_g) > nW * cap:
            raise AssertionError(f"class {g} over capacity: {len(S_g)}")
        win = _assign_windows(D_g, nW, cap, rng)
        fail = win < 0
        for i in np.flatnonzero(fail):
            ovf_s.append(np.repeat(S_g[i], g))
            ovf_d.append(D_g[i])
        for j, wi in enumerate(wins):
            m = win == j
            fill_window(wi, S_g[m], D_g[m])

    # ---- overflow: greedy per edge over OVF windows (k=1, only dst
    # distinctness matters; sources may repeat within a window)
    ov_s = np.concatenate(ovf_s) if ovf_s else np.zeros(0, np.int64)
    ov_d = np.concatenate(ovf_d) if ovf_d else np.zeros(0, np.int64)
    nov = len(ovf_wins)
    wlists_s = [[] for _ in range(nov)]
    wlists_d = [[] for _ in range(nov)]
    wsets = [set() for _ in range(nov)]
    for s_, d_ in zip(ov_s.tolist(), ov_d.tolist()):
        for j in range(nov):
            if d_ not in wsets[j] and \
                    len(wlists_d[j]) < sched[ovf_wins[j]][0]:
                wsets[j].add(d_)
                wlists_s[j].append(s_)
                wlists_d[j].append(d_)
                break
        else:
            raise AssertionError("overflow windows exhausted")
    for j, wi in enumerate(ovf_wins):
        fill_window(wi, np.asarray(wlists_s[j], np.int64),
                    np.asarray(wlists_d[j], np.int64).reshape(-1, 1))
    return sigma, idx


def prep_inputs(cfg: Cfg, x, edge_index, batch, W1, b1, W2, b2):
    x = np.asarray(x, np.float32).reshape(-1)
    ei = np.asarray(edge_index)
    batch = np.asarray(batch).astype(np.int64)
    src = ei[0].astype(np.int64)
    dst = ei[1].astype(np.int64)

    deg = 1.0 + np.bincount(dst, minlength=cfg.N)
    dinv = (1.0 / np.sqrt(deg)).astype(np.float32)

    shard = dst // cfg.RS
    rng = np.random.default_rng(12345)
    sig_idx = []
    for sh in range(2 * cfg.NC):
        m = shard == sh
        sig_idx.append(plan_shard(src[m], dst[m] - sh * cfg.RS, rng, cfg))

    iota128 = np.tile(np.arange(P, dtype=np.float32), (P, 1))
    iotagc = np.tile(np.arange(cfg.GCOL, dtype=np.float32), (P, 1))

    def colwrap(a):
        return np.ascontiguousarray(a.reshape(-1, P).T)

    in_maps = []
    for c in range(cfg.NC):
        sigA, idxA = sig_idx[2 * c]
        sigB, idxB = sig_idx[2 * c + 1]
        xs, dvs = [], []
        for sig in (sigA, sigB):
            sg = np.where(sig >= 0, sig, 0)
            valid = (sig >= 0).astype(np.float32)
            xs.append(colwrap(x[sg] * valid))
            dvs.append(colwrap(dinv[sg] * valid))
        gn = c * cfg.RC + np.arange(cfg.RC)
        bat = batch[gn]
        idx_cat = np.concatenate([idxA, idxB])
        in_maps.append({
            "x_sa": xs[0], "dv_sa": dvs[0],
            "x_sb": xs[1], "dv_sb": dvs[1],
            "x_own": colwrap(x[gn]),
            "dv_own": colwrap(dinv[gn]),
            "dstw": np.ascontiguousarray(np.tile(idx_cat.reshape(-1, 16).T, (8, 1))),
            "plo": colwrap((bat % P).astype(np.float32)),
            "phi": colwrap((bat // P).astype(np.float32)),
            "iota128": iota128,
            "iotagc": iotagc,
            "W1": np.asarray(W1, np.float32).reshape(1, cfg.HID),
            "b1": np.asarray(b1, np.float32).reshape(cfg.HID),
            "W2": np.asarray(W2, np.float32).reshape(cfg.HID, cfg.CLS),
            "b2": np.asarray(b2, np.float32).reshape(cfg.CLS),
        })
    return in_maps


# ---------------------------------------------------------------- kernel

def _declare_io(nc, cfg: Cfg):
    f32 = mybir.dt.float32
    i16 = mybir.dt.int16
    t = {}
    t["x_sa"] = nc.dram_tensor("x_sa", [P, cfg.SIGCOLS], f32, kind="ExternalInput")
    t["dv_sa"] = nc.dram_tensor("dv_sa", [P, cfg.SIGCOLS], f32, kind="ExternalInput")
    t["x_sb"] = nc.dram_tensor("x_sb", [P, cfg.SIGCOLS], f32, kind="ExternalInput")
    t["dv_sb"] = nc.dram_tensor("dv_sb", [P, cfg.SIGCOLS], f32, kind="ExternalInput")
    t["x_own"] = nc.dram_tensor("x_own", [P, cfg.COLS], f32, kind="ExternalInput")
    t["dv_own"] = nc.dram_tensor("dv_own", [P, cfg.COLS], f32, kind="ExternalInput")
    t["dstw"] = nc.dram_tensor("dstw", [P, cfg.SLOTS * 2 // 16], i16,
                               kind="ExternalInput")
    t["plo"] = nc.dram_tensor("plo", [P, cfg.COLS], f32, kind="ExternalInput")
    t["phi"] = nc.dram_tensor("phi", [P, cfg.COLS], f32, kind="ExternalInput")
    t["iota128"] = nc.dram_tensor("iota128", [P, P], f32, kind="ExternalInput")
    t["iotagc"] = nc.dram_tensor("iotagc", [P, cfg.GCOL], f32, kind="ExternalInput")
    t["W1"] = nc.dram_tensor("W1", [1, cfg.HID], f32, kind="ExternalInput")
    t["b1"] = nc.dram_tensor("b1", [cfg.HID], f32, kind="ExternalInput")
    t["W2"] = nc.dram_tensor("W2", [cfg.HID, cfg.CLS], f32, kind="ExternalInput")
    t["b2"] = nc.dram_tensor("b2", [cfg.CLS], f32, kind="ExternalInput")
    t["out"] = nc.dram_tensor("out", [cfg.G, cfg.CLS], f32, kind="ExternalOutput")
    return t


def build_nc(cfg: Cfg, reps: int = 1, scratch: int = 16384,
             do_scatter: bool = True, do_pool: bool = True):
    """reps>1 repeats the whole body (for slope-based HW timing)."""
    f32 = mybir.dt.float32
    i16 = mybir.dt.int16
    nc = bacc.Bacc("TRN2", target_bir_lowering=False, debug=False,
                   dynamic_dma_scratch_size=scratch, num_swdge_queues=cfg.NQ)
    io = _declare_io(nc, cfg)
    sched = cfg.SCHED

    with tile.TileContext(nc) as tc:
        with (
            tc.tile_pool(name="state", bufs=1) as st,
            tc.tile_pool(name="stage", bufs=1) as sg_pool,
            tc.tile_pool(name="sbuf", bufs=8) as sb,
            tc.tile_pool(name="tail", bufs=1) as tl,
            tc.tile_pool(name="psum", bufs=1, space="PSUM") as ps,
            tc.tile_pool(name="dram", bufs=1, space="DRAM") as dr,
        ):
            # ---- persistent tiles
            q_t = [[[st.tile([P, cfg.QCOLS], f32, tag=f"q{s}{a}{par}",
                             name=f"q{s}{a}{par}")
                     for par in (0, 1)] for a in range(cfg.R_ACC)]
                   for s in (0, 1)]
            p_sig = [st.tile([P, cfg.SIGCOLS], f32, name=f"psig{s}")
                     for s in (0, 1)]

            # ---- own-node p + pooling constants (loaded once)
            xo = sg_pool.tile([P, cfg.COLS], f32, tag="xo")
            nc.sync.dma_start(out=xo[:], in_=io["x_own"][:])
            dvo = st.tile([P, cfg.COLS], f32)
            nc.sync.dma_start(out=dvo[:], in_=io["dv_own"][:])
            p_own = st.tile([P, cfg.COLS], f32)
            nc.vector.tensor_tensor(out=p_own[:], in0=xo[:], in1=dvo[:],
                                    op=mybir.AluOpType.mult)
            io128 = st.tile([P, P], f32)
            nc.sync.dma_start(out=io128[:], in_=io["iota128"][:])
            iogc = st.tile([P, cfg.GCOL], f32)
            nc.sync.dma_start(out=iogc[:], in_=io["iotagc"][:])
            plo_t = st.tile([P, cfg.COLS], f32)
            nc.sync.dma_start(out=plo_t[:], in_=io["plo"][:])
            phi_t = st.tile([P, cfg.COLS], f32)
            nc.sync.dma_start(out=phi_t[:], in_=io["phi"][:])
            ps_cnt = ps.tile([P, cfg.GCOL], f32, tag="pscnt")
            ps_sum = ps.tile([P, cfg.GCOL], f32, tag="pssum")
            s_d = st.tile([P, cfg.COLS], f32, name="s_d")

            for _rep in range(reps):
                for s in (0, 1):
                    for a in range(cfg.R_ACC):
                        for par in (0, 1):
                            nc.vector.memzero(q_t[s][a][par][:])

                # ---- p_sigma per shard
                for s, (xd, dd) in enumerate(((io["x_sa"], io["dv_sa"]),
                                              (io["x_sb"], io["dv_sb"]))):
                    xt = sg_pool.tile([P, cfg.SIGCOLS], f32, tag="xs")
                    nc.sync.dma_start(out=xt[:], in_=xd[:])
                    dt_ = sg_pool.tile([P, cfg.SIGCOLS], f32, tag="ds")
                    nc.sync.dma_start(out=dt_[:], in_=dd[:])
                    nc.vector.tensor_tensor(out=p_sig[s][:], in0=xt[:],
                                            in1=dt_[:],
                                            op=mybir.AluOpType.mult)

                # ---- scatter stream + per-shard pooling
                call_no = 0
                for s in (0, 1):
                    ib = cfg.SLOTS * s
                    sig_col = 0
                    for wi, (w, k) in enumerate(sched):
                        wcols = w // P
                        n = w * k
                        it = sb.tile([P, 504], i16, tag="idx")
                        nc.sync.dma_start(
                            out=it[:, :n // 16],
                            in_=io["dstw"][:, (ib // 16):(ib + n) // 16])
                        if k == 1:
                            vals = p_sig[s][:, sig_col:sig_col + wcols] \
                                .rearrange("p (c o) -> p c o", o=1)
                        else:
                            msg = sb.tile([P, 64, 1], f32, tag="msg")
                            nc.scalar.activation(
                                msg[:, :k * wcols, 0]
                                    .rearrange("p (r c) -> p r c", r=k),
                                p_sig[s][:, sig_col:sig_col + wcols]
                                    .rearrange("p (o c) -> p o c", o=1)
                                    .to_broadcast([P, k, wcols]),
                                mybir.ActivationFunctionType.Copy)
                            vals = msg[:, :k * wcols, :]
                        a = call_no % cfg.R_ACC
                        if not do_scatter:
                            call_no += 1
                            ib += n
                            sig_col += wcols
                            continue
                        nc.gpsimd.dma_scatter_add(
                            q_t[s][a][0][:], vals, it[:, :n // 16],
                            n, n, 1,
                            sbuf_tokens_per_rank=P, parity_reg=0,
                            out_ap_other=q_t[s][a][1][:],
                            queue_num=call_no % cfg.NQ)
                        call_no += 1
                        ib += n
                        sig_col += wcols

                    # ---- shard tail: q dense, s, pooling
                    qg_d = tl.tile([P, cfg.COLS // 2], f32, tag=f"qg{s}")
                    PG = cfg.RS // 256    # 75 data groups
                    for par in (0, 1):
                        nc.vector.tensor_tensor(
                            out=q_t[s][0][par][:, :PG],
                            in0=q_t[s][0][par][:, :PG],
                            in1=q_t[s][1][par][:, :PG],
                            op=mybir.AluOpType.add)
                        nc.vector.tensor_tensor(
                            out=q_t[s][2][par][:, :PG],
                            in0=q_t[s][2][par][:, :PG],
                            in1=q_t[s][3][par][:, :PG],
                            op=mybir.AluOpType.add)
                        nc.vector.tensor_tensor(
                            out=qg_d[:, par:2 * PG:2]
                                .rearrange("p (c o) -> p c o", o=1),
                            in0=q_t[s][0][par][:, :PG]
                                .rearrange("p (c o) -> p c o", o=1),
                            in1=q_t[s][2][par][:, :PG]
                                .rearrange("p (c o) -> p c o", o=1),
                            op=mybir.AluOpType.add)
                    c0 = s * (cfg.COLS // 2)
                    nc.vector.tensor_tensor(
                        out=s_d[:, c0:c0 + cfg.COLS // 2],
                        in0=p_own[:, c0:c0 + cfg.COLS // 2], in1=qg_d[:],
                        op=mybir.AluOpType.add)
                    nc.vector.tensor_tensor(
                        out=s_d[:, c0:c0 + cfg.COLS // 2],
                        in0=s_d[:, c0:c0 + cfg.COLS // 2],
                        in1=dvo[:, c0:c0 + cfg.COLS // 2],
                        op=mybir.AluOpType.mult)
                    for t in range(c0, c0 + cfg.COLS // 2):
                        if not do_pool and 0 < t < cfg.COLS - 1:
                            continue
                        oh_lo = sb.tile([P, P], f32, tag="ohlo")
                        nc.vector.tensor_scalar(
                            out=oh_lo[:], in0=io128[:],
                            scalar1=plo_t[:, t:t + 1],
                            scalar2=None, op0=mybir.AluOpType.is_equal)
                        oh_s = sb.tile([P, P], f32, tag="ohs")
                        nc.vector.tensor_scalar(
                            out=oh_s[:], in0=oh_lo[:],
                            scalar1=s_d[:, t:t + 1],
                            scalar2=None, op0=mybir.AluOpType.mult)
                        oh_hi = sb.tile([P, cfg.GCOL], f32, tag="ohhi")
                        nc.vector.tensor_scalar(
                            out=oh_hi[:], in0=iogc[:],
                            scalar1=phi_t[:, t:t + 1],
                            scalar2=None, op0=mybir.AluOpType.is_equal)
                        nc.tensor.matmul(ps_cnt[:], lhsT=oh_lo[:],
                                         rhs=oh_hi[:], start=(t == 0),
                                         stop=(t == cfg.COLS - 1))
                        nc.tensor.matmul(ps_sum[:], lhsT=oh_s[:],
                                         rhs=oh_hi[:], start=(t == 0),
                                         stop=(t == cfg.COLS - 1))

                # ---- AllReduce (sum, cnt)
                g_d = tl.tile([P, cfg.GCOL, 2], f32)
                nc.vector.tensor_copy(
                    out=g_d[:, :, 0:1],
                    in_=ps_sum[:].rearrange("p (a o) -> p a o", o=1))
                nc.vector.tensor_copy(
                    out=g_d[:, :, 1:2],
                    in_=ps_cnt[:].rearrange("p (a o) -> p a o", o=1))
                r_in = dr.tile([P, cfg.GCOL * 2], f32, tag="rin")
                r_out = dr.tile([P, cfg.GCOL * 2], f32, tag="rout")
                nc.gpsimd.dma_start(out=r_in[:],
                                    in_=g_d[:].rearrange("p a b -> p (a b)"))
                nc.gpsimd.collective_compute(
                    "AllReduce", mybir.AluOpType.add,
                    replica_groups=[list(range(cfg.NC))],
                    ins=[r_in[:].opt()], outs=[r_out[:].opt()])
                sg = tl.tile([P, cfg.GCOL, 2], f32)
                nc.sync.dma_start(out=sg[:].rearrange("p a b -> p (a b)"),
                                  in_=r_out[:])

                # ---- v = W1@W2, u = b1@W2 + b2 (broadcast to 128)
                w1t = tl.tile([cfg.HID, 1], f32)
                nc.sync.dma_start(out=w1t[:],
                                  in_=io["W1"][:].rearrange("o k -> k o"))
                b1t = tl.tile([cfg.HID, 1], f32)
                nc.sync.dma_start(
                    out=b1t[:], in_=io["b1"][:].rearrange("(k o) -> k o", o=1))
                w2t = tl.tile([cfg.HID, cfg.CLS], f32)
                nc.sync.dma_start(out=w2t[:], in_=io["W2"][:])
                b2t = tl.tile([1, cfg.CLS], f32)
                nc.sync.dma_start(
                    out=b2t[:], in_=io["b2"][:].rearrange("(o k) -> o k", o=1))
                pv1 = ps.tile([1, cfg.CLS], f32, tag="pv1")
                nc.tensor.matmul(pv1[:], lhsT=w1t[:], rhs=w2t[:],
                                 start=True, stop=True)
                pu1 = ps.tile([1, cfg.CLS], f32, tag="pu1")
                nc.tensor.matmul(pu1[:], lhsT=b1t[:], rhs=w2t[:],
                                 start=True, stop=True)
                v1 = tl.tile([1, cfg.CLS], f32)
                nc.vector.tensor_copy(out=v1[:], in_=pv1[:])
                u1 = tl.tile([1, cfg.CLS], f32)
                nc.vector.tensor_tensor(out=u1[:], in0=pu1[:], in1=b2t[:],
                                        op=mybir.AluOpType.add)
                ones_row = tl.tile([1, P], f32)
                nc.vector.memset(ones_row[:], 1.0)
                pvb = ps.tile([P, cfg.CLS], f32, tag="pvb")
                nc.tensor.matmul(pvb[:], lhsT=ones_row[:], rhs=v1[:],
                                 start=True, stop=True)
                pub = ps.tile([P, cfg.CLS], f32, tag="pub")
                nc.tensor.matmul(pub[:], lhsT=ones_row[:], rhs=u1[:],
                                 start=True, stop=True)
                vb = tl.tile([P, cfg.CLS], f32)
                nc.vector.tensor_copy(out=vb[:], in_=pvb[:])
                ub = tl.tile([P, cfg.CLS], f32)
                nc.vector.tensor_copy(out=ub[:], in_=pub[:])

                # ---- t = S / max(cnt,1); logits; log_softmax
                cntc = tl.tile([P, cfg.GCOL], f32)
                nc.vector.tensor_scalar(out=cntc[:], in0=sg[:, :, 1],
                                        scalar1=1.0, scalar2=None,
                                        op0=mybir.AluOpType.max)
                rcp = tl.tile([P, cfg.GCOL], f32)
                nc.vector.reciprocal(rcp[:], cntc[:])
                tg = tl.tile([P, cfg.GCOL], f32)
                nc.vector.tensor_tensor(out=tg[:], in0=sg[:, :, 0],
                                        in1=rcp[:],
                                        op=mybir.AluOpType.mult)
                L = tl.tile([P, cfg.GCOL, cfg.CLS], f32)
                for c in range(cfg.CLS):
                    nc.vector.tensor_scalar(
                        out=L[:, :, c], in0=tg[:],
                        scalar1=vb[:, c:c + 1], scalar2=ub[:, c:c + 1],
                        op0=mybir.AluOpType.mult, op1=mybir.AluOpType.add)
                m = tl.tile([P, cfg.GCOL], f32)
                nc.vector.tensor_reduce(out=m[:], in_=L[:],
                                        axis=mybir.AxisListType.X,
                                        op=mybir.AluOpType.max)
                Lm = tl.tile([P, cfg.GCOL, cfg.CLS], f32)
                nc.vector.tensor_tensor(
                    out=Lm[:], in0=L[:],
                    in1=m[:].to_broadcast([P, cfg.GCOL, cfg.CLS]),
                    op=mybir.AluOpType.subtract)
                ex = tl.tile([P, cfg.GCOL, cfg.CLS], f32)
                nc.scalar.activation(ex[:], Lm[:],
                                     mybir.ActivationFunctionType.Exp)
                se = tl.tile([P, cfg.GCOL], f32)
                nc.vector.tensor_reduce(out=se[:], in_=ex[:],
                                        axis=mybir.AxisListType.X,
                                        op=mybir.AluOpType.add)
                ls = tl.tile([P, cfg.GCOL], f32)
                nc.scalar.activation(ls[:], se[:],
                                     mybir.ActivationFunctionType.Ln)
                outt = tl.tile([P, cfg.GCOL, cfg.CLS], f32)
                nc.vector.tensor_tensor(
                    out=outt[:], in0=Lm[:],
                    in1=ls[:].to_broadcast([P, cfg.GCOL, cfg.CLS]),
                    op=mybir.AluOpType.subtract)
                nc.sync.dma_start(
                    out=io["out"][:].rearrange("(c p) k -> p c k", p=P),
                    in_=outt[:])

    nc.compile()
    return nc


def build_noop(cfg: Cfg):
    """Same I/O signature, trivial device work — isolates host overhead."""
    f32 = mybir.dt.float32
    nc = bacc.Bacc("TRN2", target_bir_lowering=False, debug=False)
    io = _declare_io(nc, cfg)
    with tile.TileContext(nc) as tc:
        with tc.tile_pool(name="sbuf", bufs=1) as sb:
            z = sb.tile([P, cfg.GCOL, cfg.CLS], f32)
            nc.vector.memzero(z[:])
            nc.sync.dma_start(
                out=io["out"][:].rearrange("(c p) k -> p c k", p=P), in_=z[:])
    nc.compile()
    return nc


_NC_CACHE = {}


def _get_nc(cfg: Cfg):
    if cfg not in _NC_CACHE:
        _NC_CACHE[cfg] = build_nc(cfg)
    return _NC_CACHE[cfg]


def run(cfg: Cfg, inputs, **run_kwargs):
    nc = _get_nc(cfg)
    in_maps = prep_inputs(cfg, **inputs)
    res = bass_utils.run_bass_kernel_spmd(
        nc, in_maps, core_ids=list(range(cfg.NC)), **run_kwargs)
    return res


def kernel(x, edge_index, batch, W1, b1, W2, b2):
    cfg = Cfg()
    res = run(cfg, dict(x=x, edge_index=edge_index, batch=batch,
                        W1=W1, b1=b1, W2=W2, b2=b2))
    return res.results[0]["out"]


# revision 6
# speedup vs baseline: 80.3925x; 1.0897x over previous
"""GCN (gnn_message_passing) Trainium2 kernel v2, 8 NeuronCores.

Math (IN_F=1 makes GCNConv rank-1; per-node scalars):
    deg[n]  = 1 + global indegree(n)        (host, structure-only)
    dinv[n] = 1/sqrt(deg[n])                (host, structure-only)
    p[n]    = x[n] * dinv[n]                (device)
    q[n]    = sum_{e: dst=n} p[src_e]       (device scatter-add stream)
    s[n]    = dinv[n] * (p[n] + q[n])       (device; self loop included)
    t_g     = mean_{n in g} s[n]            (device one-hot matmul pooling)
    logits  = t_g*(W1@W2) + (b1@W2+b2); log_softmax.   (device)

Scatter-only message passing (no per-edge gather at all):
  - Nodes are dst-range sharded: core c owns [c*38400, (c+1)*38400), split
    into 2 shards of 19200 (dst slot ids must fit int16).
  - For each (core, shard), the host groups sources by out-degree k into
    that shard (k>=4 split into parts of <=3; multi-edges and k>=7 tails go
    to an overflow pool) and packs them into "windows" of W sources such
    that ALL dsts in a window are distinct (randomized greedy).
  - x and dinv are uploaded in this per-shard source order sigma (each
    source at most once per (core,shard,group) + tiny overflow dup).
    Device computes p_sigma = x_sigma*dinv_sigma, then a free DVE broadcast
    "expand" replicates each window's p column block K times (K = group k);
    plane r of the window holds each source's r-th edge.
  - One dma_scatter_add per window (num_idxs = K*W, all dsts distinct)
    accumulates into per-shard parity-split SBUF q tiles, round-robin over
    2 accumulator pairs and 4 SWDGE queues for pipelining.
  - Overflow pool is scheduled as levels of (unique-src, unique-dst) edges,
    k=1 style.
  - Pooling/AllReduce/tail as in v1 (one-hot matmuls, [G,10] logits).
"""

import sys
for _p in ("/opt/trn_rl_repo", "/root/.axon_site/_ro/trn_rl_repo"):
    if _p not in sys.path:
        sys.path.insert(0, _p)

from dataclasses import dataclass

import numpy as np

import concourse.bacc as bacc
import concourse.mybir as mybir
import concourse.tile as tile
from concourse import bass_utils

P = 128


@dataclass(frozen=True)
class Cfg:
    N: int = 307200          # nodes
    E: int = 5734400         # edges
    G: int = 4096            # graphs
    CLS: int = 10
    HID: int = 64
    NC: int = 8              # cores
    RS: int = 19200          # nodes per shard (2 shards per core)
    TRASH: int = 19456       # trash slot base (distinct per call position)
    QCOLS: int = 112         # q tile cols: covers idx<=27647 (g<=107)
    R_ACC: int = 4           # accumulator pairs per shard
    NQ: int = 4              # SWDGE queues
    # static per-shard window schedule: (W sources, K planes) per window
    K1W: int = 8064
    K1N: int = 15
    K2W: int = 3968
    K2N: int = 22
    K3W: int = 2688
    K3N: int = 11
    OVF: tuple = (2560, 1536, 1024, 512, 256, 256, 128, 128)

    @property
    def RC(self):
        return 2 * self.RS

    @property
    def COLS(self):          # free cols of dense per-core node tiles
        return self.RC // P

    @property
    def GCOL(self):
        return self.G // P

    @property
    def SCHED(self):
        s = [(self.K1W, 1)] * self.K1N + [(self.K2W, 2)] * self.K2N \
            + [(self.K3W, 3)] * self.K3N + [(w, 1) for w in self.OVF]
        return tuple(s)

    @property
    def SIGCOLS(self):       # sigma cols per shard
        return sum(w // P for w, _ in self.SCHED)

    @property
    def SLOTS(self):         # idx slots per shard
        return sum(w * k for w, k in self.SCHED)


# ---------------------------------------------------------------- planner

def _assign_windows(D, nW, cap, rng, rounds=24):
    """D [n, k] dst-local ids. Place each row into one of nW windows of
    capacity cap with all dsts distinct within a window (across rows and
    planes). Returns win [n] (-1 = failed)."""
    n, k = D.shape
    RS = 19200
    win = np.full(n, -1, np.int64)
    fill = np.zeros(nW, np.int64)
    used = np.zeros(nW * RS, bool)
    pending = rng.permutation(n)
    for _ in range(rounds):
        if len(pending) == 0:
            break
        cand = rng.integers(0, nW, len(pending))
        notfull = fill[cand] < cap
        cand2 = rng.integers(0, nW, len(pending))
        cand = np.where(notfull, cand, cand2)
        keys = (cand[:, None] * RS + D[pending]).ravel()
        uniq, first = np.unique(keys, return_index=True)
        pos = np.searchsorted(uniq, keys)
        is_first = first[pos] == np.arange(len(keys))
        ok = (is_first & ~used[keys]).reshape(len(pending), k).all(1)
        # capacity: rank within window among batch + fill < cap
        sel_rows = np.flatnonzero(ok)
        if len(sel_rows):
            sw = cand[sel_rows]
            o = np.argsort(sw, kind="stable")
            ss = sw[o]
            gf = np.ones(len(ss), bool)
            gf[1:] = ss[1:] != ss[:-1]
            firsts = np.flatnonzero(gf)
            gidx = np.cumsum(gf) - 1
            rank = np.arange(len(ss)) - firsts[gidx]
            keep = np.empty(len(sw), bool)
            keep[o] = rank + fill[ss] < cap
            ok[sel_rows[~keep]] = False
        sel = pending[ok]
        wsel = cand[ok]
        win[sel] = wsel
        used[(wsel[:, None] * RS + D[sel]).ravel()] = True
        np.add.at(fill, wsel, 1)
        pending = pending[~ok]
    return win


def plan_shard(src_sh, dl_sh, rng, cfg: Cfg):
    """Schedule one (core, shard)'s edges.

    Returns (sigma [SIGCOLS*128] int64 source ids (-1 pad),
             idx   [SLOTS] int16 dst slots)."""
    RS = cfg.RS
    # ---- dedup multi-edges: keep first (s,d), rest -> overflow
    key = src_sh * RS + dl_sh
    order = np.argsort(key, kind="stable")
    ks = key[order]
    fs = np.ones(len(ks), bool)
    fs[1:] = ks[1:] != ks[:-1]
    main_s, main_d = src_sh[order][fs], dl_sh[order][fs]
    ovf_s = [src_sh[order][~fs]]
    ovf_d = [dl_sh[order][~fs]]

    k_of = np.bincount(main_s, minlength=cfg.N)
    o2 = np.argsort(main_s, kind="stable")
    dsort = main_d[o2]
    off = np.zeros(cfg.N + 1, np.int64)
    np.cumsum(k_of, out=off[1:])

    # ---- build part lists per class g: S_g sources, D_g [m, g] dsts
    parts_S = {1: [], 2: [], 3: []}
    parts_D = {1: [], 2: [], 3: []}
    SPLIT = {1: (1,), 2: (2,), 3: (3,), 4: (2, 2), 5: (3, 2), 6: (3, 3)}
    for k in range(1, 7):
        S = np.flatnonzero(k_of == k)
        if not len(S):
            continue
        D = dsort[off[S][:, None] + np.arange(k)[None, :]]
        c0 = 0
        for g in SPLIT[k]:
            parts_S[g].append(S)
            parts_D[g].append(D[:, c0:c0 + g])
            c0 += g
    S = np.flatnonzero(k_of > 6)
    for s in S:
        ovf_s.append(np.full(k_of[s], s))
        ovf_d.append(dsort[off[s]:off[s + 1]])

    # ---- greedy window assignment per class
    sigma = np.full(cfg.SIGCOLS * P, -1, np.int64)
    idx = np.zeros(cfg.SLOTS, np.int16)
    sched = cfg.SCHED
    # window slot ranges
    sig_base = np.zeros(len(sched), np.int64)
    idx_base = np.zeros(len(sched), np.int64)
    sb = ib = 0
    for i, (w, k) in enumerate(sched):
        sig_base[i], idx_base[i] = sb, ib
        sb += w
        ib += w * k
    win_of_class = {
        1: list(range(0, cfg.K1N)),
        2: list(range(cfg.K1N, cfg.K1N + cfg.K2N)),
        3: list(range(cfg.K1N + cfg.K2N, cfg.K1N + cfg.K2N + cfg.K3N)),
    }
    ovf_wins = list(range(cfg.K1N + cfg.K2N + cfg.K3N, len(sched)))

    def fill_window(wi, S_sel, D_sel):
        """Place rows (sources S_sel with dsts D_sel [m,g]) into window wi."""
        w, k = sched[wi]
        m = len(S_sel)
        assert m <= w and D_sel.shape[1] == k
        sigma[sig_base[wi]:sig_base[wi] + m] = S_sel
        blk = np.full((k, w), -1, np.int64)
        blk[:, :m] = D_sel.T
        pad = blk < 0
        blk[pad] = cfg.TRASH + (np.arange(k * w).reshape(k, w))[pad] % 13312
        idx[idx_base[wi]:idx_base[wi] + w * k] = blk.reshape(-1)

    for g in (1, 2, 3):
        if not parts_S[g]:
            for wi in win_of_class[g]:
                fill_window(wi, np.zeros(0, np.int64),
                            np.zeros((0, g), np.int64))
            continue
        S_g = np.concatenate(parts_S[g])
        D_g = np.concatenate(parts_D[g])
        wins = win_of_class[g]
        nW, cap = len(wins), sched[wins[0]][0]
        if len(S_g) > nW * cap:
            raise AssertionError(f"class {g} over capacity: {len(S_g)}")
        win = _assign_windows(D_g, nW, cap, rng)
        fail = win < 0
        for i in np.flatnonzero(fail):
            ovf_s.append(np.repeat(S_g[i], g))
            ovf_d.append(D_g[i])
        for j, wi in enumerate(wins):
            m = win == j
            fill_window(wi, S_g[m], D_g[m])

    # ---- overflow: greedy per edge over OVF windows (k=1, only dst
    # distinctness matters; sources may repeat within a window)
    ov_s = np.concatenate(ovf_s) if ovf_s else np.zeros(0, np.int64)
    ov_d = np.concatenate(ovf_d) if ovf_d else np.zeros(0, np.int64)
    nov = len(ovf_wins)
    wlists_s = [[] for _ in range(nov)]
    wlists_d = [[] for _ in range(nov)]
    wsets = [set() for _ in range(nov)]
    for s_, d_ in zip(ov_s.tolist(), ov_d.tolist()):
        for j in range(nov):
            if d_ not in wsets[j] and \
                    len(wlists_d[j]) < sched[ovf_wins[j]][0]:
                wsets[j].add(d_)
                wlists_s[j].append(s_)
                wlists_d[j].append(d_)
                break
        else:
            raise AssertionError("overflow windows exhausted")
    for j, wi in enumerate(ovf_wins):
        fill_window(wi, np.asarray(wlists_s[j], np.int64),
                    np.asarray(wlists_d[j], np.int64).reshape(-1, 1))
    return sigma, idx


def prep_inputs(cfg: Cfg, x, edge_index, batch, W1, b1, W2, b2):
    x = np.asarray(x, np.float32).reshape(-1)
    ei = np.asarray(edge_index)
    batch = np.asarray(batch).astype(np.int64)
    src = ei[0].astype(np.int64)
    dst = ei[1].astype(np.int64)

    deg = 1.0 + np.bincount(dst, minlength=cfg.N)
    dinv = (1.0 / np.sqrt(deg)).astype(np.float32)

    shard = dst // cfg.RS
    rng = np.random.default_rng(12345)
    sig_idx = []
    for sh in range(2 * cfg.NC):
        m = shard == sh
        sig_idx.append(plan_shard(src[m], dst[m] - sh * cfg.RS, rng, cfg))

    iota128 = np.tile(np.arange(P, dtype=np.float32), (P, 1))
    iotagc = np.tile(np.arange(cfg.GCOL, dtype=np.float32), (P, 1))

    def colwrap(a):
        return np.ascontiguousarray(a.reshape(-1, P).T)

    in_maps = []
    for c in range(cfg.NC):
        sigA, idxA = sig_idx[2 * c]
        sigB, idxB = sig_idx[2 * c + 1]
        xs, dvs = [], []
        for sig in (sigA, sigB):
            sg = np.where(sig >= 0, sig, 0)
            valid = (sig >= 0).astype(np.float32)
            xs.append(colwrap(x[sg] * valid))
            dvs.append(colwrap(dinv[sg] * valid))
        gn = c * cfg.RC + np.arange(cfg.RC)
        bat = batch[gn]
        idx_cat = np.concatenate([idxA, idxB])
        in_maps.append({
            "x_sa": xs[0], "dv_sa": dvs[0],
            "x_sb": xs[1], "dv_sb": dvs[1],
            "x_own": colwrap(x[gn]),
            "dv_own": colwrap(dinv[gn]),
            "dstw": np.ascontiguousarray(np.tile(idx_cat.reshape(-1, 16).T, (8, 1))),
            "plo": colwrap((bat % P).astype(np.float32)),
            "phi": colwrap((bat // P).astype(np.float32)),
            "iota128": iota128,
            "iotagc": iotagc,
            "W1": np.asarray(W1, np.float32).reshape(1, cfg.HID),
            "b1": np.asarray(b1, np.float32).reshape(cfg.HID),
            "W2": np.asarray(W2, np.float32).reshape(cfg.HID, cfg.CLS),
            "b2": np.asarray(b2, np.float32).reshape(cfg.CLS),
        })
    return in_maps


# ---------------------------------------------------------------- kernel

def _declare_io(nc, cfg: Cfg):
    f32 = mybir.dt.float32
    i16 = mybir.dt.int16
    t = {}
    t["x_sa"] = nc.dram_tensor("x_sa", [P, cfg.SIGCOLS], f32, kind="ExternalInput")
    t["dv_sa"] = nc.dram_tensor("dv_sa", [P, cfg.SIGCOLS], f32, kind="ExternalInput")
    t["x_sb"] = nc.dram_tensor("x_sb", [P, cfg.SIGCOLS], f32, kind="ExternalInput")
    t["dv_sb"] = nc.dram_tensor("dv_sb", [P, cfg.SIGCOLS], f32, kind="ExternalInput")
    t["x_own"] = nc.dram_tensor("x_own", [P, cfg.COLS], f32, kind="ExternalInput")
    t["dv_own"] = nc.dram_tensor("dv_own", [P, cfg.COLS], f32, kind="ExternalInput")
    t["dstw"] = nc.dram_tensor("dstw", [P, cfg.SLOTS * 2 // 16], i16,
                               kind="ExternalInput")
    t["plo"] = nc.dram_tensor("plo", [P, cfg.COLS], f32, kind="ExternalInput")
    t["phi"] = nc.dram_tensor("phi", [P, cfg.COLS], f32, kind="ExternalInput")
    t["iota128"] = nc.dram_tensor("iota128", [P, P], f32, kind="ExternalInput")
    t["iotagc"] = nc.dram_tensor("iotagc", [P, cfg.GCOL], f32, kind="ExternalInput")
    t["W1"] = nc.dram_tensor("W1", [1, cfg.HID], f32, kind="ExternalInput")
    t["b1"] = nc.dram_tensor("b1", [cfg.HID], f32, kind="ExternalInput")
    t["W2"] = nc.dram_tensor("W2", [cfg.HID, cfg.CLS], f32, kind="ExternalInput")
    t["b2"] = nc.dram_tensor("b2", [cfg.CLS], f32, kind="ExternalInput")
    t["out"] = nc.dram_tensor("out", [cfg.G, cfg.CLS], f32, kind="ExternalOutput")
    return t


def build_nc(cfg: Cfg, reps: int = 1, scratch: int = 32768,
             do_scatter: bool = True, do_pool: bool = True):
    """reps>1 repeats the whole body (for slope-based HW timing)."""
    f32 = mybir.dt.float32
    i16 = mybir.dt.int16
    nc = bacc.Bacc("TRN2", target_bir_lowering=False, debug=False,
                   dynamic_dma_scratch_size=scratch, num_swdge_queues=cfg.NQ)
    io = _declare_io(nc, cfg)
    sched = cfg.SCHED

    with tile.TileContext(nc) as tc:
        with (
            tc.tile_pool(name="state", bufs=1) as st,
            tc.tile_pool(name="stage", bufs=1) as sg_pool,
            tc.tile_pool(name="sbuf", bufs=8) as sb,
            tc.tile_pool(name="tail", bufs=1) as tl,
            tc.tile_pool(name="psum", bufs=1, space="PSUM") as ps,
            tc.tile_pool(name="dram", bufs=1, space="DRAM") as dr,
        ):
            # ---- persistent tiles
            q_t = [[[st.tile([P, cfg.QCOLS], f32, tag=f"q{s}{a}{par}",
                             name=f"q{s}{a}{par}")
                     for par in (0, 1)] for a in range(cfg.R_ACC)]
                   for s in (0, 1)]
            p_sig = [st.tile([P, cfg.SIGCOLS], f32, name=f"psig{s}")
                     for s in (0, 1)]

            # ---- own-node p + pooling constants (loaded once)
            xo = sg_pool.tile([P, cfg.COLS], f32, tag="xo")
            nc.sync.dma_start(out=xo[:], in_=io["x_own"][:])
            dvo = st.tile([P, cfg.COLS], f32)
            nc.sync.dma_start(out=dvo[:], in_=io["dv_own"][:])
            p_own = st.tile([P, cfg.COLS], f32)
            nc.vector.tensor_tensor(out=p_own[:], in0=xo[:], in1=dvo[:],
                                    op=mybir.AluOpType.mult)
            io128 = st.tile([P, P], f32)
            nc.sync.dma_start(out=io128[:], in_=io["iota128"][:])
            iogc = st.tile([P, cfg.GCOL], f32)
            nc.sync.dma_start(out=iogc[:], in_=io["iotagc"][:])
            plo_t = st.tile([P, cfg.COLS], f32)
            nc.sync.dma_start(out=plo_t[:], in_=io["plo"][:])
            phi_t = st.tile([P, cfg.COLS], f32)
            nc.sync.dma_start(out=phi_t[:], in_=io["phi"][:])
            ps_cnt = ps.tile([P, cfg.GCOL], f32, tag="pscnt")
            ps_sum = ps.tile([P, cfg.GCOL], f32, tag="pssum")
            s_d = st.tile([P, cfg.COLS], f32, name="s_d")

            for _rep in range(reps):
                for s in (0, 1):
                    for a in range(cfg.R_ACC):
                        for par in (0, 1):
                            nc.vector.memzero(q_t[s][a][par][:])

                # ---- p_sigma per shard
                for s, (xd, dd) in enumerate(((io["x_sa"], io["dv_sa"]),
                                              (io["x_sb"], io["dv_sb"]))):
                    xt = sg_pool.tile([P, cfg.SIGCOLS], f32, tag="xs")
                    nc.sync.dma_start(out=xt[:], in_=xd[:])
                    dt_ = sg_pool.tile([P, cfg.SIGCOLS], f32, tag="ds")
                    nc.sync.dma_start(out=dt_[:], in_=dd[:])
                    nc.vector.tensor_tensor(out=p_sig[s][:], in0=xt[:],
                                            in1=dt_[:],
                                            op=mybir.AluOpType.mult)

                # ---- scatter stream + per-shard pooling
                call_no = 0
                for s in (0, 1):
                    ib = cfg.SLOTS * s
                    sig_col = 0
                    for wi, (w, k) in enumerate(sched):
                        wcols = w // P
                        n = w * k
                        it = sb.tile([P, 504], i16, tag="idx")
                        nc.sync.dma_start(
                            out=it[:, :n // 16],
                            in_=io["dstw"][:, (ib // 16):(ib + n) // 16])
                        if k == 1:
                            vals = p_sig[s][:, sig_col:sig_col + wcols] \
                                .rearrange("p (c o) -> p c o", o=1)
                        else:
                            msg = sb.tile([P, 64, 1], f32, tag="msg")
                            nc.scalar.activation(
                                msg[:, :k * wcols, 0]
                                    .rearrange("p (r c) -> p r c", r=k),
                                p_sig[s][:, sig_col:sig_col + wcols]
                                    .rearrange("p (o c) -> p o c", o=1)
                                    .to_broadcast([P, k, wcols]),
                                mybir.ActivationFunctionType.Copy)
                            vals = msg[:, :k * wcols, :]
                        a = call_no % cfg.R_ACC
                        if not do_scatter:
                            call_no += 1
                            ib += n
                            sig_col += wcols
                            continue
                        nc.gpsimd.dma_scatter_add(
                            q_t[s][a][0][:], vals, it[:, :n // 16],
                            n, n, 1,
                            sbuf_tokens_per_rank=P, parity_reg=0,
                            out_ap_other=q_t[s][a][1][:],
                            queue_num=call_no % cfg.NQ)
                        call_no += 1
                        ib += n
                        sig_col += wcols

                    # ---- shard tail: q dense, s, pooling
                    qg_d = tl.tile([P, cfg.COLS // 2], f32, tag=f"qg{s}")
                    PG = cfg.RS // 256    # 75 data groups
                    for par in (0, 1):
                        nc.vector.tensor_tensor(
                            out=q_t[s][0][par][:, :PG],
                            in0=q_t[s][0][par][:, :PG],
                            in1=q_t[s][1][par][:, :PG],
                            op=mybir.AluOpType.add)
                        nc.vector.tensor_tensor(
                            out=q_t[s][2][par][:, :PG],
                            in0=q_t[s][2][par][:, :PG],
                            in1=q_t[s][3][par][:, :PG],
                            op=mybir.AluOpType.add)
                        nc.vector.tensor_tensor(
                            out=qg_d[:, par:2 * PG:2]
                                .rearrange("p (c o) -> p c o", o=1),
                            in0=q_t[s][0][par][:, :PG]
                                .rearrange("p (c o) -> p c o", o=1),
                            in1=q_t[s][2][par][:, :PG]
                                .rearrange("p (c o) -> p c o", o=1),
                            op=mybir.AluOpType.add)
                    c0 = s * (cfg.COLS // 2)
                    nc.vector.tensor_tensor(
                        out=s_d[:, c0:c0 + cfg.COLS // 2],
                        in0=p_own[:, c0:c0 + cfg.COLS // 2], in1=qg_d[:],
                        op=mybir.AluOpType.add)
                    nc.vector.tensor_tensor(
                        out=s_d[:, c0:c0 + cfg.COLS // 2],
                        in0=s_d[:, c0:c0 + cfg.COLS // 2],
                        in1=dvo[:, c0:c0 + cfg.COLS // 2],
                        op=mybir.AluOpType.mult)
                    for t in range(c0, c0 + cfg.COLS // 2):
                        if not do_pool and 0 < t < cfg.COLS - 1:
                            continue
                        oh_lo = sb.tile([P, P], f32, tag="ohlo")
                        nc.vector.tensor_scalar(
                            out=oh_lo[:], in0=io128[:],
                            scalar1=plo_t[:, t:t + 1],
                            scalar2=None, op0=mybir.AluOpType.is_equal)
                        oh_s = sb.tile([P, P], f32, tag="ohs")
                        nc.vector.tensor_scalar(
                            out=oh_s[:], in0=oh_lo[:],
                            scalar1=s_d[:, t:t + 1],
                            scalar2=None, op0=mybir.AluOpType.mult)
                        oh_hi = sb.tile([P, cfg.GCOL], f32, tag="ohhi")
                        nc.vector.tensor_scalar(
                            out=oh_hi[:], in0=iogc[:],
                            scalar1=phi_t[:, t:t + 1],
                            scalar2=None, op0=mybir.AluOpType.is_equal)
                        nc.tensor.matmul(ps_cnt[:], lhsT=oh_lo[:],
                                         rhs=oh_hi[:], start=(t == 0),
                                         stop=(t == cfg.COLS - 1))
                        nc.tensor.matmul(ps_sum[:], lhsT=oh_s[:],
                                         rhs=oh_hi[:], start=(t == 0),
                                         stop=(t == cfg.COLS - 1))

                # ---- AllReduce (sum, cnt)
                g_d = tl.tile([P, cfg.GCOL, 2], f32)
                nc.vector.tensor_copy(
                    out=g_d[:, :, 0:1],
                    in_=ps_sum[:].rearrange("p (a o) -> p a o", o=1))
                nc.vector.tensor_copy(
                    out=g_d[:, :, 1:2],
                    in_=ps_cnt[:].rearrange("p (a o) -> p a o", o=1))
                r_in = dr.tile([P, cfg.GCOL * 2], f32, tag="rin")
                r_out = dr.tile([P, cfg.GCOL * 2], f32, tag="rout")
                nc.gpsimd.dma_start(out=r_in[:],
                                    in_=g_d[:].rearrange("p a b -> p (a b)"))
                nc.gpsimd.collective_compute(
                    "AllReduce", mybir.AluOpType.add,
                    replica_groups=[list(range(cfg.NC))],
                    ins=[r_in[:].opt()], outs=[r_out[:].opt()])
                sg = tl.tile([P, cfg.GCOL, 2], f32)
                nc.sync.dma_start(out=sg[:].rearrange("p a b -> p (a b)"),
                                  in_=r_out[:])

                # ---- v = W1@W2, u = b1@W2 + b2 (broadcast to 128)
                w1t = tl.tile([cfg.HID, 1], f32)
                nc.sync.dma_start(out=w1t[:],
                                  in_=io["W1"][:].rearrange("o k -> k o"))
                b1t = tl.tile([cfg.HID, 1], f32)
                nc.sync.dma_start(
                    out=b1t[:], in_=io["b1"][:].rearrange("(k o) -> k o", o=1))
                w2t = tl.tile([cfg.HID, cfg.CLS], f32)
                nc.sync.dma_start(out=w2t[:], in_=io["W2"][:])
                b2t = tl.tile([1, cfg.CLS], f32)
                nc.sync.dma_start(
                    out=b2t[:], in_=io["b2"][:].rearrange("(o k) -> o k", o=1))
                pv1 = ps.tile([1, cfg.CLS], f32, tag="pv1")
                nc.tensor.matmul(pv1[:], lhsT=w1t[:], rhs=w2t[:],
                                 start=True, stop=True)
                pu1 = ps.tile([1, cfg.CLS], f32, tag="pu1")
                nc.tensor.matmul(pu1[:], lhsT=b1t[:], rhs=w2t[:],
                                 start=True, stop=True)
                v1 = tl.tile([1, cfg.CLS], f32)
                nc.vector.tensor_copy(out=v1[:], in_=pv1[:])
                u1 = tl.tile([1, cfg.CLS], f32)
                nc.vector.tensor_tensor(out=u1[:], in0=pu1[:], in1=b2t[:],
                                        op=mybir.AluOpType.add)
                ones_row = tl.tile([1, P], f32)
                nc.vector.memset(ones_row[:], 1.0)
                pvb = ps.tile([P, cfg.CLS], f32, tag="pvb")
                nc.tensor.matmul(pvb[:], lhsT=ones_row[:], rhs=v1[:],
                                 start=True, stop=True)
                pub = ps.tile([P, cfg.CLS], f32, tag="pub")
                nc.tensor.matmul(pub[:], lhsT=ones_row[:], rhs=u1[:],
                                 start=True, stop=True)
                vb = tl.tile([P, cfg.CLS], f32)
                nc.vector.tensor_copy(out=vb[:], in_=pvb[:])
                ub = tl.tile([P, cfg.CLS], f32)
                nc.vector.tensor_copy(out=ub[:], in_=pub[:])

                # ---- t = S / max(cnt,1); logits; log_softmax
                cntc = tl.tile([P, cfg.GCOL], f32)
                nc.vector.tensor_scalar(out=cntc[:], in0=sg[:, :, 1],
                                        scalar1=1.0, scalar2=None,
                                        op0=mybir.AluOpType.max)
                rcp = tl.tile([P, cfg.GCOL], f32)
                nc.vector.reciprocal(rcp[:], cntc[:])
                tg = tl.tile([P, cfg.GCOL], f32)
                nc.vector.tensor_tensor(out=tg[:], in0=sg[:, :, 0],
                                        in1=rcp[:],
                                        op=mybir.AluOpType.mult)
                L = tl.tile([P, cfg.GCOL, cfg.CLS], f32)
                for c in range(cfg.CLS):
                    nc.vector.tensor_scalar(
                        out=L[:, :, c], in0=tg[:],
                        scalar1=vb[:, c:c + 1], scalar2=ub[:, c:c + 1],
                        op0=mybir.AluOpType.mult, op1=mybir.AluOpType.add)
                m = tl.tile([P, cfg.GCOL], f32)
                nc.vector.tensor_reduce(out=m[:], in_=L[:],
                                        axis=mybir.AxisListType.X,
                                        op=mybir.AluOpType.max)
                Lm = tl.tile([P, cfg.GCOL, cfg.CLS], f32)
                nc.vector.tensor_tensor(
                    out=Lm[:], in0=L[:],
                    in1=m[:].to_broadcast([P, cfg.GCOL, cfg.CLS]),
                    op=mybir.AluOpType.subtract)
                ex = tl.tile([P, cfg.GCOL, cfg.CLS], f32)
                nc.scalar.activation(ex[:], Lm[:],
                                     mybir.ActivationFunctionType.Exp)
                se = tl.tile([P, cfg.GCOL], f32)
                nc.vector.tensor_reduce(out=se[:], in_=ex[:],
                                        axis=mybir.AxisListType.X,
                                        op=mybir.AluOpType.add)
                ls = tl.tile([P, cfg.GCOL], f32)
                nc.scalar.activation(ls[:], se[:],
                                     mybir.ActivationFunctionType.Ln)
                outt = tl.tile([P, cfg.GCOL, cfg.CLS], f32)
                nc.vector.tensor_tensor(
                    out=outt[:], in0=Lm[:],
                    in1=ls[:].to_broadcast([P, cfg.GCOL, cfg.CLS]),
                    op=mybir.AluOpType.subtract)
                nc.sync.dma_start(
                    out=io["out"][:].rearrange("(c p) k -> p c k", p=P),
                    in_=outt[:])

    nc.compile()
    return nc


def build_noop(cfg: Cfg):
    """Same I/O signature, trivial device work — isolates host overhead."""
    f32 = mybir.dt.float32
    nc = bacc.Bacc("TRN2", target_bir_lowering=False, debug=False)
    io = _declare_io(nc, cfg)
    with tile.TileContext(nc) as tc:
        with tc.tile_pool(name="sbuf", bufs=1) as sb:
            z = sb.tile([P, cfg.GCOL, cfg.CLS], f32)
            nc.vector.memzero(z[:])
            nc.sync.dma_start(
                out=io["out"][:].rearrange("(c p) k -> p c k", p=P), in_=z[:])
    nc.compile()
    return nc


_NC_CACHE = {}


def _get_nc(cfg: Cfg):
    if cfg not in _NC_CACHE:
        _NC_CACHE[cfg] = build_nc(cfg)
    return _NC_CACHE[cfg]


def run(cfg: Cfg, inputs, **run_kwargs):
    nc = _get_nc(cfg)
    in_maps = prep_inputs(cfg, **inputs)
    res = bass_utils.run_bass_kernel_spmd(
        nc, in_maps, core_ids=list(range(cfg.NC)), **run_kwargs)
    return res


def kernel(x, edge_index, batch, W1, b1, W2, b2):
    cfg = Cfg()
    res = run(cfg, dict(x=x, edge_index=edge_index, batch=batch,
                        W1=W1, b1=b1, W2=W2, b2=b2))
    return res.results[0]["out"]
